# revision 11
# baseline (speedup 1.0000x reference)
"""AsymFormer forward on 8 TRN2 NeuronCores — data-parallel over batch.

v3 design:
 - B=8 -> one batch element per core, no collectives.
 - Relation branch (phase 1): 65536 rel-rows per core.
   * a-side: rel-row chunks (27 feats x 128 rows) ride as matmul lhsT
     (stationary); rhs = folded 27->32 projection. Output lands directly in
     (k-partition, hc) layout -> ZERO transposes.
   * variance: packed (108, 512) tiles (4 chunks of 27 feats) -> one
     block-diag Cholesky matmul + ACT Square + a data-as-lhsT reduction
     matmul (ycsq.T @ o27sel) gives variances already row-major.
   * sigma via ACT Sqrt (tiny, 16/partition), 1/sigma via Pool pow(x,-1),
     normalize via DVE mult with stride-0 broadcast AP, +bias on Pool.
 - Phase 2: attention R-add via PE PSUM-accumulation (identity matmul of
   R_T slices); conn-mul on DVE pair-batched; Exp pair-batched; softmax
   denominators via whole-tile reciprocal + gpsimd partition_broadcast;
   LN 1/sigma via Pool pow; exact gelu replaced by tanh-form built from
   Square/Tanh (same ACT table set as Exp -> no act-table switches).
 - bf16 throughout the bandwidth/compute-heavy paths; residual jf in f32.
"""

import os
import sys

sys.path.insert(0, "/opt/trn_rl_repo")

KDBG = os.environ.get("KDBG", "")

import numpy as np

import concourse.bacc as bacc
import concourse.bass as bass
import concourse.mybir as mybir
import concourse.tile as tile
from concourse.bass_utils import run_bass_kernel_spmd

B, N, C, H, DP = 8, 256, 128, 8, 4
HS = C // H  # 16
SCALE = 0.25
NN = N * N
F32 = mybir.dt.float32
BF16 = mybir.dt.bfloat16
F32R = mybir.dt.float32r
NPBF = mybir.dt.np(BF16)
AF = mybir.ActivationFunctionType
ALU = mybir.AluOpType
GELC0 = 0.7978845608028654
GELC1 = 0.044715

NG = 32          # phase-1 groups (2048 rel-rows each)
DP_EMIT = int(os.environ.get("DP_EMIT", DP))

last_results = None


def _r(ap):
    return ap.bitcast(F32R)


def _fold(inp):
    f = lambda k: np.asarray(inp[k], np.float32)
    w = {}
    # relation encoder collapse 26->128
    Wc = f("re_w1") @ f("re_w2") @ f("re_w3")
    bc = (f("re_b1") @ f("re_w2") + f("re_b2")) @ f("re_w3") + f("re_b3")
    P = np.eye(128, dtype=np.float64) - 1.0 / 128.0
    Mh = np.concatenate(
        [P @ Wc.T.astype(np.float64), P @ bc.astype(np.float64).reshape(128, 1)],
        axis=1,
    )
    G = Mh.T @ Mh
    Rc = np.linalg.cholesky(G + 1e-14 * np.eye(27)).T  # upper, Rc.T@Rc = G
    Rc = Rc.astype(np.float32)
    # block-diag for packed yc matmul: (108, 128), [27g+f, 32g+r] = Rc[r, f]
    RcBD = np.zeros((108, 128), np.float32)
    for g in range(4):
        RcBD[27 * g : 27 * g + 27, 32 * g : 32 * g + 27] = Rc.T
    w["RcBD"] = RcBD.astype(NPBF)
    # o27sel (128, 4): [32g+r, g] = 1/128 for r < 27
    o27 = np.zeros((128, 4), np.float32)
    for g in range(4):
        o27[32 * g : 32 * g + 27, g] = 1.0 / 128.0
    w["o27sel"] = o27.astype(NPBF)
    # ln2+SCALE fold into rconv -> Wr2 (128, 32), mean-centered
    Wr = np.empty((128, DP, H), np.float32)
    br = np.empty((DP, H), np.float32)
    for i in range(DP):
        Wr[:, i, :] = SCALE * (f("ln2_g")[i][:, None] * f("rconv_w")[i])
        br[i] = SCALE * (f("ln2_b")[i] @ f("rconv_w")[i] + f("rconv_b")[i])
    Wr2 = Wr.reshape(128, DP * H)
    Wr2 = Wr2 - np.ones((128, 1), np.float32) * (Wr2.sum(0, keepdims=True) / 128.0)
    WqA = np.concatenate([Wc @ Wr2, (Wr2.T @ bc).reshape(1, 32)], axis=0)  # (27,32)
    w["WqA"] = WqA.astype(NPBF)
    brO = np.broadcast_to(br.reshape(-1), (128, 2, 8, 32))
    w["brO"] = np.ascontiguousarray(brO).astype(NPBF)
    # joint encoder collapse 96->128
    Wj = f("je_w1") @ f("je_w2") @ f("je_w3")
    bj = (f("je_b1") @ f("je_w2") + f("je_b2")) @ f("je_w3") + f("je_b3")
    w["Wj"] = np.ascontiguousarray(Wj)
    w["bj"] = np.ascontiguousarray(bj.reshape(128, 1))
    # per-depth: ln1 into qkv (+SCALE on q), ln3 into mw1
    qkvw = np.empty((DP, C, 3 * C), np.float32)
    qkvb = np.empty((DP, 3 * C), np.float32)
    mw1 = np.empty((DP, C, C), np.float32)
    mb1 = np.empty((DP, C), np.float32)
    for i in range(DP):
        qkvw[i] = f("ln1_g")[i][:, None] * f("qkv_w")[i]
        qkvb[i] = f("ln1_b")[i] @ f("qkv_w")[i] + f("qkv_b")[i]
        qkvw[i][:, :C] *= SCALE
        qkvb[i][:C] *= SCALE
        mw1[i] = f("ln3_g")[i][:, None] * f("mw1")[i]
        mb1[i] = f("ln3_b")[i] @ f("mw1")[i] + f("mb1")[i]
    # qk: heads 4u+j at 32j+s (s<16) within tile u
    qkw = np.zeros((C, DP, 2, 2, C), np.float32)
    qkb = np.zeros((C, DP, 2, 2), np.float32)
    for i in range(DP):
        for t in range(2):
            wt = qkvw[i][:, t * C : (t + 1) * C]
            bt = qkvb[i][t * C : (t + 1) * C]
            for h in range(H):
                u, j = divmod(h, 4)
                qkw[:, i, t, u, 32 * j : 32 * j + HS] = wt[:, h * HS : (h + 1) * HS]
                qkb[32 * j : 32 * j + HS, i, t, u] = bt[h * HS : (h + 1) * HS]
    w["qkw"] = qkw.astype(NPBF)
    w["qkb"] = np.ascontiguousarray(qkb)
    w["vw"] = np.ascontiguousarray(qkvw.transpose(1, 0, 2)[:, :, 2 * C :]).astype(NPBF)
    w["vb"] = np.ascontiguousarray(qkvb[:, 2 * C :].T)
    # proj grouped by head PAIRS: (64, DP, 4, C)
    projP = np.zeros((64, DP, 4, C), np.float32)
    for i in range(DP):
        for h in range(H):
            p, j = divmod(h, 2)
            projP[32 * j : 32 * j + HS, i, p, :] = f("proj_w")[i][
                h * HS : (h + 1) * HS, :
            ]
    w["projP"] = projP.astype(NPBF)
    w["projb"] = np.ascontiguousarray(f("proj_b").T)
    w["mw1"] = np.ascontiguousarray(mw1.transpose(1, 0, 2)).astype(NPBF)
    w["mb1"] = np.ascontiguousarray(mb1.T)
    w["mw2h"] = np.ascontiguousarray(0.5 * f("mw2").transpose(1, 0, 2)).astype(NPBF)
    w["mb2"] = np.ascontiguousarray(f("mb2").T)
    # decoder with final LN affine folded
    Wdc = f("dw1") @ f("dw2") @ f("dw3")
    Wd = f("ng")[:, None] * Wdc
    bd = f("nb") @ Wdc + (f("db1") @ f("dw2") + f("db2")) @ f("dw3") + f("db3")
    w["Wd"] = np.ascontiguousarray(Wd).astype(NPBF)
    w["bd"] = np.ascontiguousarray(bd.reshape(90, 1))
    w["idb"] = np.eye(128, dtype=np.float32).astype(NPBF)
    w["idf"] = np.eye(128, dtype=np.float32)
    return w


def _build():
    nc = bacc.Bacc(None, target_bir_lowering=False)

    def din(name, shape, dt=F32):
        return nc.dram_tensor(name, list(shape), dt, kind="ExternalInput")

    relq_d = din("relq", (8, 27, 8192), BF16)
    relp_d = din("relp", (8, 108, 2048), BF16)
    connS2_d = din("connS2", (128, 2, 2, N), BF16)
    jT_d = din("jT", (96, N), F32R)
    RcBD_d = din("RcBD", (108, 128), BF16)
    WqA_d = din("WqA", (27, 32), BF16)
    o27_d = din("o27sel", (128, 4), BF16)
    brO_d = din("brO", (128, 2, 8, 32), BF16)
    Wj_d = din("Wj", (96, 128), F32R)
    bj_d = din("bj", (128, 1))
    qkw_d = din("qkw", (C, DP, 2, 2, C), BF16)
    qkb_d = din("qkb", (C, DP, 2, 2))
    vw_d = din("vw", (C, DP, C), BF16)
    vb_d = din("vb", (C, DP))
    projP_d = din("projP", (64, DP, 4, C), BF16)
    projb_d = din("projb", (C, DP))
    mw1_d = din("mw1", (C, DP, C), BF16)
    mb1_d = din("mb1", (C, DP))
    mw2h_d = din("mw2h", (C, DP, C), BF16)
    mb2_d = din("mb2", (C, DP))
    Wd_d = din("Wd", (128, 90), BF16)
    bd_d = din("bd", (90, 1))
    idb_d = din("idb", (128, 128), BF16)
    idf_d = din("idf", (128, 128))
    out_d = nc.dram_tensor("out", [N, 90], F32, kind="ExternalOutput")
    if KDBG:
        rtd_d = nc.dram_tensor("rt_dbg", [128, 2, N, 32], BF16, kind="ExternalOutput")
        jf_d = nc.dram_tensor("jf_dbg", [2, 128, 128], F32, kind="ExternalOutput")
        pr_d = nc.dram_tensor("pr_dbg", [128, N], BF16, kind="ExternalOutput")
        og_d = nc.dram_tensor("og_dbg", [4, 64, N], BF16, kind="ExternalOutput")
        xt_d = nc.dram_tensor("xt_dbg", [128, N], BF16, kind="ExternalOutput")

    from contextlib import ExitStack

    with tile.TileContext(nc) as tc, ExitStack() as ctx, nc.allow_low_precision(
        reason="bf16 pipeline; end-to-end precision checked in test"
    ):
        const = ctx.enter_context(tc.tile_pool(name="const", bufs=1))
        zin = ctx.enter_context(tc.tile_pool(name="zin", bufs=2))
        st = ctx.enter_context(tc.tile_pool(name="st", bufs=6))
        wrk = ctx.enter_context(tc.tile_pool(name="wrk", bufs=3))
        wrk4 = ctx.enter_context(tc.tile_pool(name="wrk4", bufs=4))
        psm = ctx.enter_context(tc.tile_pool(name="psm", bufs=4, space="PSUM"))
        psm2 = ctx.enter_context(tc.tile_pool(name="psm2", bufs=2, space="PSUM"))

        def cload(dt_handle, shape, tag, dt=F32):
            t = const.tile(list(shape), dt, tag=tag)
            nc.scalar.dma_start(out=t, in_=dt_handle[:])
            return t

        RcBD_s = cload(RcBD_d, (108, 128), "RcBD", BF16)
        WqA_s = cload(WqA_d, (27, 32), "WqA", BF16)
        o27_s = cload(o27_d, (128, 4), "o27sel", BF16)
        brO_s = cload(brO_d, (128, 2, 8, 32), "brO", BF16)
        eps_s = const.tile([128, 1], F32, tag="eps")
        nc.vector.memset(eps_s[:], 1e-5)
        mone_s = const.tile([128, 2, 2, 4], F32, tag="mone")
        nc.vector.memset(mone_s[:], -1.0)
        mh1_s = const.tile([128, 1], F32, tag="mh1")
        nc.vector.memset(mh1_s[:], -0.5)

        Wj_s = cload(Wj_d, (96, 128), "Wj", F32R)
        bj_s = cload(bj_d, (128, 1), "bj")
        qkw_s = cload(qkw_d, (C, DP, 2, 2, C), "qkw", BF16)
        qkb_s = cload(qkb_d, (C, DP, 2, 2), "qkb")
        vw_s = cload(vw_d, (C, DP, C), "vw", BF16)
        vb_s = cload(vb_d, (C, DP), "vb")
        projP_s = cload(projP_d, (64, DP, 4, C), "projP", BF16)
        projb_s = cload(projb_d, (C, DP), "projb")
        mw1_s = cload(mw1_d, (C, DP, C), "mw1", BF16)
        mb1_s = cload(mb1_d, (C, DP), "mb1")
        mw2h_s = cload(mw2h_d, (C, DP, C), "mw2h", BF16)
        mb2_s = cload(mb2_d, (C, DP), "mb2")
        Wd_s = cload(Wd_d, (128, 90), "Wd", BF16)
        bd_s = cload(bd_d, (90, 1), "bd")
        idb_s = cload(idb_d, (128, 128), "idb", BF16)
        idf_s = cload(idf_d, (128, 128), "idf")
        connS2_s = cload(connS2_d, (128, 2, 2, N), "connS2", BF16)
        jT_s = cload(jT_d, (96, N), "jT", F32R)

        R_T = const.tile([128, 2, N, 32], BF16, tag="R_T")
        # attention head-pair oTn tiles; junk rows zeroed once
        og = []
        for p4 in range(4):
            t = const.tile([64, N], BF16, tag=f"og{p4}")
            nc.vector.memset(t[:], 0.0)
            og.append(t)
        # v_ext tiles: ones col 32, zeros cols 16..31, set once
        vxt = []
        for kt in range(2):
            t = const.tile([128, H, 33], BF16, tag=f"vx{kt}")
            nc.vector.memset(t[:, :, 16:33], 0.0)
            nc.vector.memset(t[:, :, 32:33], 1.0)
            vxt.append(t)

        # ---------------- Phase 1: relation branch -> R_T ----------------
        def p1_A(Dj, relq, relp):
            D, j = Dj
            yc_ps = psm.tile([128, 512], F32, tag="b")
            nc.tensor.matmul(
                yc_ps, RcBD_s[:], relp[:, j * 512 : (j + 1) * 512],
                start=True, stop=True,
            )
            a_ps = psm.tile([128, 2, 2, 4, 32], F32, tag="b")
            for m in range(16):
                kt, dq = m % 2, m // 2
                qq, g = dq // 4, dq % 4
                nc.tensor.matmul(
                    a_ps[:, kt, qq, g, :],
                    relq[:, j * 2048 + m * 128 : j * 2048 + (m + 1) * 128],
                    WqA_s[:], start=True, stop=True,
                )
            return yc_ps, a_ps

        def p1_B(Dj, yc_ps, a_ps):
            ycsq = wrk.tile([128, 512], BF16, tag="ycsq")
            nc.scalar.activation(ycsq, yc_ps, AF.Square)
            var_q = psm.tile([128, 2, 2, 4], F32, tag="b")
            for s in range(4):
                nc.tensor.matmul(
                    var_q[:, s % 2, s // 2, :],
                    ycsq[:, s * 128 : (s + 1) * 128],
                    o27_s[:], start=True, stop=True,
                )
            sg = st.tile([128, 2, 2, 4], F32, tag="sg")
            nc.scalar.activation(sg, var_q, AF.Sqrt, bias=eps_s[:])
            rsg = st.tile([128, 2, 2, 4, 1], F32, tag="rsg")
            nc.gpsimd.tensor_tensor(
                out=rsg.rearrange("p a b c d -> p a b (c d)"),
                in0=sg, in1=mone_s, op=ALU.pow,
            )
            return a_ps, rsg

        def p1_C(Dj, a_ps, rsg):
            D, j = Dj
            G = 4 * D + j
            out = R_T[:, :, 8 * G : 8 * G + 8, :].rearrange(
                "p kt (qq g) h -> p kt qq g h", qq=2
            )
            ia, ib = bass.broadcast_tensor_aps(a_ps[:], rsg[:])
            nc.vector.tensor_tensor(out=out, in0=ia, in1=ib, op=ALU.mult)

        def p1_D(Dj):
            D, j = Dj
            G = 4 * D + j
            sl = R_T[:, :, 8 * G : 8 * G + 8, :]
            nc.gpsimd.tensor_tensor(out=sl, in0=sl, in1=brO_s[:], op=ALU.add)

        q1, q2 = [], []
        for D in range(8):
            relq = zin.tile([27, 8192], BF16, tag="relq")
            nc.sync.dma_start(out=relq, in_=relq_d[D])
            relp = zin.tile([108, 2048], BF16, tag="relp")
            nc.sync.dma_start(out=relp, in_=relp_d[D])
            for j in range(4):
                q1.append(((D, j), p1_A((D, j), relq, relp)))
                if len(q1) > 1:
                    Dj, (yc, ap) = q1.pop(0)
                    q2.append((Dj, p1_B(Dj, yc, ap)))
                if len(q2) > 1:
                    Dj, (ap, rsg) = q2.pop(0)
                    p1_C(Dj, ap, rsg)
                    p1_D(Dj)
        for Dj, (yc, ap) in q1:
            q2.append((Dj, p1_B(Dj, yc, ap)))
        for Dj, (ap, rsg) in q2:
            p1_C(Dj, ap, rsg)
            p1_D(Dj)

        # ---------------- joint encoder -> jf (token-major) --------------
        jf = []
        jp = psm.tile([128, N], F32, tag="b")
        nc.tensor.matmul(jp, Wj_s[:], jT_s[:], start=True, stop=True)
        jfT = wrk.tile([128, N], F32, tag="jfT")
        nc.scalar.activation(jfT, jp, AF.Identity, bias=bj_s[:])
        for qt in range(2):
            jft = const.tile([128, 128], F32, tag=f"jf{qt}")
            tp = psm.tile([128, 128], F32, tag="b")
            nc.tensor.transpose(tp, jfT[:, qt * 128 : (qt + 1) * 128], idf_s[:])
            nc.vector.tensor_copy(out=jft[:], in_=tp[:])
            jf.append(jft)

        def layer_norm_t(tag):
            """token-major standardize -> feature-major (128, 256) bf16."""
            xT = wrk.tile([128, N], BF16, tag=f"xT{tag}")
            for qt in range(2):
                st6 = st.tile([128, 6], F32, tag="st6")
                nc.vector.bn_stats(out=st6, in_=jf[qt][:])
                mv = st.tile([128, 2], F32, tag="mv")
                nc.vector.bn_aggr(out=mv, in_=st6[:])
                rs0 = st.tile([128, 1], F32, tag="rs0")
                nc.gpsimd.tensor_tensor(
                    out=rs0, in0=mv[:, 1:2], in1=mh1_s[:], op=ALU.pow
                )
                # one Newton step: rs = rs0*(1.5 - 0.5*var*rs0^2)
                y2 = st.tile([128, 1], F32, tag="y2")
                nc.vector.tensor_tensor(out=y2, in0=rs0, in1=rs0, op=ALU.mult)
                hv = st.tile([128, 1], F32, tag="hv")
                nc.vector.tensor_scalar(
                    out=hv, in0=mv[:, 1:2], scalar1=-0.5, scalar2=None, op0=ALU.mult
                )
                w15 = st.tile([128, 1], F32, tag="w15")
                nc.vector.tensor_scalar(
                    out=w15, in0=y2, scalar1=hv[:], scalar2=1.5,
                    op0=ALU.mult, op1=ALU.add,
                )
                rs1 = st.tile([128, 1], F32, tag="rs1")
                nc.vector.tensor_tensor(out=rs1, in0=rs0, in1=w15, op=ALU.mult)
                xh = wrk4.tile([128, 128], BF16, tag="xh")
                nc.vector.tensor_scalar(
                    out=xh, in0=jf[qt][:], scalar1=mv[:, 0:1], scalar2=rs1[:],
                    op0=ALU.subtract, op1=ALU.mult,
                )
                tp = psm.tile([128, 128], BF16, tag="b")
                nc.tensor.transpose(tp, xh[:], idb_s[:])
                nc.scalar.activation(xT[:, qt * 128 : (qt + 1) * 128], tp, AF.Identity)
            return xT

        # ---------------- Phase 2: transformer depths ---------------------
        for i in range(DP_EMIT):
            xT = layer_norm_t(f"1_{i}")
            if KDBG and i == 0:
                nc.sync.dma_start(out=xt_d[:], in_=xT[:])
            qkT = [[None, None], [None, None]]
            for t in range(2):
                for u in range(2):
                    ps = psm.tile([128, N], F32, tag="b")
                    nc.tensor.matmul(
                        ps, qkw_s[:, i, t, u, :], xT[:], start=True, stop=True
                    )
                    sb = wrk.tile([128, N], BF16, tag=f"qk{t}{u}")
                    nc.scalar.activation(
                        sb, ps, AF.Identity, bias=qkb_s[:, i, t, u : u + 1]
                    )
                    qkT[t][u] = sb
            vps = psm.tile([128, N], F32, tag="b")
            nc.tensor.matmul(vps, vw_s[:, i, :], xT[:], start=True, stop=True)
            vT = wrk.tile([128, N], BF16, tag="vT")
            nc.scalar.activation(vT, vps, AF.Identity, bias=vb_s[:, i : i + 1])
            for kt in range(2):
                vtp = psm.tile([128, 128], BF16, tag="b")
                nc.tensor.transpose(vtp, vT[:, kt * 128 : (kt + 1) * 128], idb_s[:])
                nc.vector.tensor_copy(
                    out=vxt[kt][:, :, 0:16],
                    in_=vtp.rearrange("p (h c) -> p h c", h=H),
                )

            for p4 in range(4):
                sp2 = psm2.tile([128, 2, 2, N], F32, tag="sp")
                for j2 in range(2):
                    h = 2 * p4 + j2
                    u, j4 = divmod(h, 4)
                    hp = slice(32 * j4, 32 * j4 + HS)
                    ih = i * 8 + h
                    for kt in range(2):
                        nc.tensor.matmul(
                            sp2[:, j2, kt, :],
                            qkT[1][u][hp, kt * 128 : (kt + 1) * 128],
                            qkT[0][u][hp, :],
                            start=True, stop=False, tile_position=(32 * j4, 0),
                        )
                        nc.tensor.matmul(
                            sp2[:, j2, kt, :], idb_s[:], R_T[:, kt, :, ih],
                            start=False, stop=True,
                        )
                pl2 = wrk4.tile([128, 2, 2, N], BF16, tag="pl2")
                nc.vector.tensor_tensor(
                    out=pl2, in0=sp2, in1=connS2_s[:], op=ALU.mult
                )
                Ek2 = wrk4.tile([128, 2, 2, N], BF16, tag="Ek2")
                nc.scalar.activation(Ek2, pl2, AF.Exp)
                o2 = psm.tile([128, N], F32, tag="b")
                for j2 in range(2):
                    h = 2 * p4 + j2
                    for kt in range(2):
                        nc.tensor.matmul(
                            o2[64 * j2 : 64 * j2 + 33, :],
                            vxt[kt][:, h, :], Ek2[:, j2, kt, :],
                            start=(kt == 0), stop=(kt == 1),
                            tile_position=(0, 64 * j2),
                        )
                for j2 in range(2):
                    dv = st.tile([1, N], F32, tag="dv")
                    nc.vector.reciprocal(
                        out=dv, in_=o2[64 * j2 + 32 : 64 * j2 + 33, :]
                    )
                    dh16 = st.tile([16, N], F32, tag="dh16")
                    nc.gpsimd.partition_broadcast(dh16, dv)
                    nc.vector.tensor_tensor(
                        out=og[p4][32 * j2 : 32 * j2 + 16, :],
                        in0=o2[64 * j2 : 64 * j2 + 16, :],
                        in1=dh16, op=ALU.mult,
                    )
            pr = psm.tile([128, N], F32, tag="b")
            for p4 in range(4):
                nc.tensor.matmul(
                    pr, projP_s[:, i, p4, :], og[p4][:],
                    start=(p4 == 0), stop=(p4 == 3),
                )
            prT = wrk.tile([128, N], BF16, tag="prT")
            nc.scalar.activation(prT, pr, AF.Identity, bias=projb_s[:, i : i + 1])
            if KDBG and i == 0:
                nc.sync.dma_start(out=pr_d[:], in_=prT[:])
                for _p in range(4):
                    nc.sync.dma_start(out=og_d[_p], in_=og[_p][:])
            for qt in range(2):
                tp = psm.tile([128, 128], BF16, tag="b")
                nc.tensor.transpose(tp, prT[:, qt * 128 : (qt + 1) * 128], idb_s[:])
                nc.vector.tensor_tensor(
                    out=jf[qt][:], in0=jf[qt][:], in1=tp, op=ALU.add
                )

            xT3 = layer_norm_t(f"3_{i}")
            h1p = psm.tile([128, N], F32, tag="b")
            nc.tensor.matmul(h1p, mw1_s[:, i, :], xT3[:], start=True, stop=True)
            xg = wrk.tile([128, N], F32, tag="xg")
            nc.scalar.activation(xg, h1p, AF.Identity, bias=mb1_s[:, i : i + 1])
            x2 = wrk4.tile([128, N], BF16, tag="x2")
            nc.scalar.activation(x2, xg, AF.Square)
            t1g = wrk4.tile([128, N], BF16, tag="t1g")
            nc.vector.tensor_scalar(
                out=t1g, in0=x2, scalar1=GELC1, scalar2=1.0,
                op0=ALU.mult, op1=ALU.add,
            )
            ug = wrk4.tile([128, N], BF16, tag="ug")
            nc.gpsimd.tensor_tensor(out=ug, in0=xg, in1=t1g, op=ALU.mult)
            tg = wrk4.tile([128, N], BF16, tag="tg")
            nc.scalar.activation(tg, ug, AF.Tanh, scale=GELC0)
            xt2 = wrk4.tile([128, N], BF16, tag="xt2")
            nc.gpsimd.tensor_tensor(out=xt2, in0=xg, in1=tg, op=ALU.mult)
            h1g = wrk4.tile([128, N], BF16, tag="h1g")
            nc.gpsimd.tensor_tensor(out=h1g, in0=xg, in1=xt2, op=ALU.add)
            h2p = psm.tile([128, N], F32, tag="b")
            nc.tensor.matmul(h2p, mw2h_s[:, i, :], h1g[:], start=True, stop=True)
            h2 = wrk.tile([128, N], BF16, tag="h2")
            nc.scalar.activation(h2, h2p, AF.Identity, bias=mb2_s[:, i : i + 1])
            for qt in range(2):
                tp = psm.tile([128, 128], BF16, tag="b")
                nc.tensor.transpose(tp, h2[:, qt * 128 : (qt + 1) * 128], idb_s[:])
                nc.vector.tensor_tensor(
                    out=jf[qt][:], in0=jf[qt][:], in1=tp, op=ALU.add
                )

        if KDBG:
            nc.sync.dma_start(out=rtd_d[:], in_=R_T[:])
            for qt in range(2):
                nc.sync.dma_start(out=jf_d[qt], in_=jf[qt][:])

        # ---------------- decoder ----------------------------------------
        xTf = layer_norm_t("f")
        op_ps = psm.tile([90, N], F32, tag="b")
        nc.tensor.matmul(op_ps, Wd_s[:], xTf[:], start=True, stop=True)
        outT = wrk.tile([90, N], F32, tag="outT")
        nc.scalar.activation(outT, op_ps, AF.Identity, bias=bd_s[:])
        for qt in range(2):
            tp = psm.tile([128, 90], F32, tag="b")
            nc.tensor.transpose(
                tp, outT[:, qt * 128 : (qt + 1) * 128], idf_s[:90, :90]
            )
            of = wrk4.tile([128, 90], F32, tag="of")
            nc.scalar.activation(of, tp, AF.Identity)
            nc.sync.dma_start(out=out_d[qt * 128 : (qt + 1) * 128, :], in_=of[:])

    nc.compile()
    return nc


def kernel(**inputs):
    global last_results
    w = _fold(inputs)
    rel = np.asarray(inputs["relation_in"], np.float32)
    conn = np.asarray(inputs["conn"], np.float32)
    joint = np.asarray(inputs["joint_in"], np.float32)

    in_maps = []
    for b in range(B):
        m = dict(w)
        flat = np.empty((27, NN), np.float32)
        flat[0:26] = rel[b].reshape(NN, 26).T
        flat[26] = 1.0
        m["relq"] = np.ascontiguousarray(
            flat.reshape(27, 8, 8192).transpose(1, 0, 2)
        ).astype(NPBF)
        # packed: [G, 27g+f, qq*256+k] = rel[b, 8G+4qq+g, k, f]
        X = rel[b].transpose(2, 0, 1).reshape(26, 32, 2, 4, 256)
        big = np.empty((32, 4, 27, 512), np.float32)
        big[:, :, 0:26, :] = X.transpose(1, 3, 0, 2, 4).reshape(32, 4, 26, 512)
        big[:, :, 26, :] = 1.0
        m["relp"] = np.ascontiguousarray(
            big.reshape(32, 108, 512).reshape(8, 4, 108, 512).transpose(0, 2, 1, 3)
        ).reshape(8, 108, 2048).astype(NPBF)
        kk = conn[b].T.reshape(2, 128, N)  # (kt, k, q)
        m["connS2"] = np.ascontiguousarray(
            np.broadcast_to(kk.transpose(1, 0, 2)[:, None, :, :], (128, 2, 2, N))
        ).astype(NPBF)
        m["jT"] = np.ascontiguousarray(joint[b].T)
        in_maps.append(m)

    nc = _build()
    last_results = run_bass_kernel_spmd(nc, in_maps, core_ids=list(range(B)))
    out = np.stack([r["out"] for r in last_results.results])
    return out.astype(np.float32)
